# revision 34
# baseline (speedup 1.0000x reference)
"""Distributed 2-layer GAT kernel for 8 Trainium2 NeuronCores (v2).

Strategy (host graph preprocessing + device SPMD kernel):
  * Nodes are relabeled by in-degree (ascending) and padded to 20480 ids.
    Blocks of 128 consecutive ids have near-uniform in-degree; the 160
    blocks are dealt round-robin to 8 cores, so every core runs the same
    per-block degree schedule ghat[l] (compile-time constant SPMD program).
  * Layer-1 table row (256 bf16 cols, 512B) = [LN(x) 128 | a_src logits
    4xf32 | pad]. W1 is applied AFTER aggregation (linearity of the GAT
    sum), which shrinks the AllGather 2.5x and the gather rows 2.5x vs
    shipping x@W1.
  * Layer-2 table row = [h@W2 128 | a_src logit f32 | pad].
  * Edge slots are dst-major; one dma_gather per 128-dst block pulls all
    in-edge rows; softmax + weighted sum via PSUM-accumulated identity
    matmuls; epilogues (1/den, bias, LN, gelu, log_softmax) are batched
    across blocks to minimize DVE instruction count.
  * AllGather outputs are addr_space="Shared" (HBM-pair buffers).
  * Pad edge slots point to table row 0 with a_src forced to -1e9 on the
    host (alsfix), making exp(leaky_relu(...)) == 0 exactly.
"""
import sys

sys.path.insert(0, "/opt/trn_rl_repo")

import numpy as np
import ml_dtypes

from concourse import bass, bacc, tile, mybir
from concourse import bass_utils
from concourse.masks import make_identity

BF16 = ml_dtypes.bfloat16
F32 = mybir.dt.float32
BF = mybir.dt.bfloat16
I16 = mybir.dt.int16
AF = mybir.ActivationFunctionType
OP = mybir.AluOpType

# problem constants
N, E = 20000, 320000
D_IN, HID, D_OUT = 128, 128, 32
H1, H2 = 4, 1
EPS = 1e-5

NCORES = 8
P = 128
NPAD = 20480            # padded node count: 160 blocks of 128
NBLK_G = NPAD // P      # 160 global blocks
NPB = NPAD // NCORES    # 2560 nodes per core
NBLK = NPB // P         # 20 blocks per core
NEG = -1e9

TCOLS = 256             # both tables: 256 bf16 cols = 512B rows
KW = 12                 # weight-multiply chunk (SBUF bound)
NAG = 4                 # AllGather chunks
GB = NBLK // NAG        # blocks per phase-2 epilogue group (5)
NSWQ = 4                # SWDGE queues

# colconst column layout (f32, replicated on all 128 partitions)
CC_GIN, CC_BIN = 0, 128
CC_G1, CC_B1, CC_BIAS1 = 256, 768, 1280
CC_G2, CC_B2, CC_BIAS2 = 1792, 1920, 2048
CC_BO = 2176
NCC = 2208


def _tid(n):
    """table row id of padded-node id n: AllGather chunk j holds blocks
    [5j, 5j+5) of every core, rank-major within the chunk."""
    blk = n // P
    c = blk % NCORES
    l = blk // NCORES
    cb = NBLK // NAG                     # blocks per AG chunk
    return ((l // cb) * (NPAD // NAG) + c * (NPB // NAG)
            + (l % cb) * P + n % P)


def prepare_inputs(x, edge_index):
    """Host graph preprocessing -> per-core arrays + degree schedule."""
    x = np.asarray(x, dtype=np.float32)
    ei = np.asarray(edge_index)
    src = np.concatenate([ei[0], np.arange(N, dtype=ei.dtype)]).astype(np.int64)
    dst = np.concatenate([ei[1], np.arange(N, dtype=ei.dtype)]).astype(np.int64)

    deg = np.bincount(dst, minlength=N)
    order = np.argsort(deg, kind="stable")        # orig node ids, deg ascending
    newid = np.empty(N, dtype=np.int64)           # orig -> padded id
    newid[order] = np.arange(N) + (NPAD - N)      # pads occupy ids [0, 480)

    degp = np.zeros(NPAD, dtype=np.int64)
    degp[newid] = deg
    gmax = degp.reshape(NBLK_G, P).max(axis=1)
    ghat = gmax.reshape(NBLK, NCORES).max(axis=1)         # per local block idx
    S = int(P * ghat.sum())                                # slots per core

    # CSR of in-edges keyed by new dst id
    nd = newid[dst]
    csr_order = np.argsort(nd, kind="stable")
    nsrc_sorted = newid[src[csr_order]]
    indptr = np.zeros(NPAD + 1, dtype=np.int64)
    np.cumsum(np.bincount(nd, minlength=NPAD), out=indptr[1:])

    # table-1 layout: single-shot AllGather output is rank-major
    blkg = np.arange(NPAD) // P
    tid1_of = (blkg % NCORES) * NPB + (blkg // NCORES) * P + np.arange(NPAD) % P
    # table-2 layout: chunked AllGather (chunk-major, rank-major within chunk)
    tid2_of = _tid(np.arange(NPAD))

    goff = np.zeros(NBLK, dtype=np.int64)                  # k-slot offsets
    goff[1:] = np.cumsum(ghat)[:-1]

    idxw1 = np.zeros((NCORES, P, S // 16), dtype=np.int16)
    idxw2 = np.zeros((NCORES, P, S // 16), dtype=np.int16)
    x_own = np.zeros((NCORES, NPB, D_IN), dtype=np.float32)
    alsfix = np.zeros((NCORES, NPB, 8), dtype=np.float32)

    inv_new = np.full(NPAD, -1, dtype=np.int64)
    inv_new[newid] = np.arange(N)

    for c in range(NCORES):
        gblk = np.arange(NBLK) * NCORES + c                # global block ids
        nid = (gblk[:, None] * P + np.arange(P)).reshape(-1)   # [NPB] padded id
        ov = inv_new[nid]                                  # orig node or -1
        real = ov >= 0
        x_own[c][real] = x[ov[real]]
        alsfix[c][~real, :] = NEG

        src_flat = np.zeros(S, dtype=np.int64)             # dummy -> node 0
        for l in range(NBLK):
            d0 = nid[l * P:(l + 1) * P]                    # padded ids of block
            base = goff[l] * P
            for p in range(P):
                d = d0[p]
                s0, s1 = indptr[d], indptr[d + 1]
                ks = np.arange(s1 - s0)
                src_flat[base + ks * P + p] = nsrc_sorted[s0:s1]
        for idxw, tid_of in ((idxw1, tid1_of), (idxw2, tid2_of)):
            idx_flat = tid_of[src_flat].astype(np.int16)
            idxw[c] = np.tile(idx_flat.reshape(S // 16, 16).T, (NCORES, 1))

    return {
        "ghat": [int(g) for g in ghat],
        "S": S,
        "idxw1": idxw1,
        "idxw2": idxw2,
        "x_own": x_own,
        "alsfix": alsfix,
        "newid": newid,
    }


def prepare_weights(W1, att1_s, att1_d, bias1, g1, b1, g_in, b_in,
                    W2, att2_s, att2_d, bias2, g2, b2, Wo, bo):
    W1 = np.asarray(W1, np.float32)
    W2 = np.asarray(W2, np.float32)
    W1h = W1.reshape(D_IN, H1, HID)
    w1a8 = np.zeros((D_IN, 8), dtype=BF16)
    w1a8[:, 0:4] = np.einsum("khc,hc->kh", W1h, np.asarray(att1_s, np.float32))
    w1a8[:, 4:8] = np.einsum("khc,hc->kh", W1h, np.asarray(att1_d, np.float32))
    w1f = W1.astype(BF16)                                  # [128, 512]

    w2e = np.zeros((4 * HID, 130), dtype=np.float32)
    w2e[:, :128] = W2
    w2e[:, 128] = W2 @ np.asarray(att2_s, np.float32)[0]
    w2e[:, 129] = W2 @ np.asarray(att2_d, np.float32)[0]
    # pack [512, 130] -> [128, 4, 130] (partition p holds rows p, 128+p, ...)
    w2ext = np.ascontiguousarray(
        w2e.reshape(4, P, 130).transpose(1, 0, 2)).astype(BF16)

    woext = np.asarray(Wo, np.float32).astype(BF16)

    cc = np.zeros(NCC, dtype=np.float32)
    cc[CC_GIN:CC_GIN + 128] = g_in
    cc[CC_BIN:CC_BIN + 128] = b_in
    cc[CC_G1:CC_G1 + 512] = g1
    cc[CC_B1:CC_B1 + 512] = b1
    cc[CC_BIAS1:CC_BIAS1 + 512] = bias1
    cc[CC_G2:CC_G2 + 128] = g2
    cc[CC_B2:CC_B2 + 128] = b2
    cc[CC_BIAS2:CC_BIAS2 + 128] = bias2
    cc[CC_BO:CC_BO + 32] = bo
    colconst = np.tile(cc[None, :], (P, 1))

    return {"w1a8": w1a8, "w1f": w1f, "w2ext": w2ext.reshape(P, 4 * 130),
            "woext": woext, "colconst": colconst}


def _bap(ap, dims):
    """AP with explicit free-dim [step, count] pairs (partition dim kept)."""
    return bass.AP(ap.tensor, ap.offset, [ap.ap[0]] + [list(d) for d in dims])


def build_program(ghat, num_devices=NCORES):
    GMAX = int(max(ghat))
    goff = np.zeros(NBLK, dtype=np.int64)
    goff[1:] = np.cumsum(ghat)[:-1]
    S = int(P * sum(ghat))

    nc = bacc.Bacc("TRN2", target_bir_lowering=False, debug=False,
                   num_devices=num_devices, num_swdge_queues=NSWQ)

    x_own = nc.dram_tensor("x_own", [NPB, D_IN], F32, kind="ExternalInput")
    idxw1 = nc.dram_tensor("idxw1", [P, S // 16], I16, kind="ExternalInput")
    idxw2 = nc.dram_tensor("idxw2", [P, S // 16], I16, kind="ExternalInput")
    alsfix = nc.dram_tensor("alsfix", [NPB, 8], F32, kind="ExternalInput")
    w1a8 = nc.dram_tensor("w1a8", [D_IN, 8], BF, kind="ExternalInput")
    w1f = nc.dram_tensor("w1f", [D_IN, 512], BF, kind="ExternalInput")
    w2ext = nc.dram_tensor("w2ext", [P, 4 * 130], BF, kind="ExternalInput")
    woext = nc.dram_tensor("woext", [P, D_OUT], BF, kind="ExternalInput")
    colconst = nc.dram_tensor("colconst", [P, NCC], F32, kind="ExternalInput")
    out = nc.dram_tensor("out", [NPB, D_OUT], F32, kind="ExternalOutput")

    rg = [list(range(num_devices))]
    qrr = [0]
    gsem = [nc.alloc_semaphore(f"gsem{q}") for q in range(NSWQ)]
    qcnt = [0] * NSWQ

    with tile.TileContext(nc) as tc:
        with (
            tc.tile_pool(name="cst", bufs=1) as cst,
            tc.tile_pool(name="wp", bufs=2) as wp,
            tc.tile_pool(name="ep", bufs=1) as ep,
            tc.tile_pool(name="gp", bufs=3) as gp,
            tc.tile_pool(name="ps", bufs=2, space="PSUM") as ps,
            tc.tile_pool(name="pss", bufs=2, space="PSUM") as pss,
            tc.tile_pool(name="dram", bufs=1, space="DRAM") as dram,
        ):
            # ---- constants ----
            for q in range(NSWQ):
                nc.gpsimd.sem_clear(gsem[q])
            ident = cst.tile([P, P], BF)
            make_identity(nc, ident[:])
            wa8 = cst.tile([P, 8], BF)
            nc.sync.dma_start(wa8[:], w1a8[:])
            w1s = cst.tile([P, 512], BF)
            nc.sync.dma_start(w1s[:], w1f[:])
            w2s = cst.tile([P, 4, 130], BF)
            nc.sync.dma_start(w2s[:], w2ext[:])
            wos = cst.tile([P, D_OUT], BF)
            nc.sync.dma_start(wos[:], woext[:])
            cc = cst.tile([P, NCC], F32)
            nc.sync.dma_start(cc[:], colconst[:])
            idx_sb1 = cst.tile([P, S // 16], I16)
            nc.sync.dma_start(idx_sb1[:], idxw1[:])
            idx_sb2 = cst.tile([P, S // 16], I16)
            nc.sync.dma_start(idx_sb2[:], idxw2[:])
            afix = cst.tile([P, NBLK, 8], F32)
            nc.sync.dma_start(
                afix[:], bass.AP(alsfix.ap().tensor, 0,
                                 [[8, P], [8 * P, NBLK], [1, 8]]))
            eps_t = cst.tile([P, 1], F32)
            nc.vector.memset(eps_t[:], EPS)

            # persistent state
            aa_all = cst.tile([P, NBLK, 8], F32)    # als(4) | ald(4) per block
            den1 = cst.tile([P, NBLK, H1], F32)
            ald2 = cst.tile([P, NBLK, 1], F32)
            den2 = cst.tile([P, NBLK, 1], F32)
            h1_all = cst.tile([P, NBLK, 512], BF)   # pre-epilogue h1 (20KB)
            h2_all = cst.tile([P, NBLK, 128], F32)  # pre-epilogue h2 (10KB)
            z_all = cst.tile([P, NBLK, D_OUT], F32)
            tcat = cst.tile([P, NBLK, TCOLS], BF)   # L1 table staging (10KB)

            # tiny dummy collective issued first: absorbs the ~35us
            # collective-stream init while phase 0 computes
            warm_in = dram.tile([8, TCOLS], BF)
            warm_out = dram.tile([8 * NCORES, TCOLS], BF)
            nc.gpsimd.collective_compute(
                "AllGather", OP.bypass, replica_groups=rg,
                ins=[warm_in[:, :].opt()], outs=[warm_out[:, :].opt()])

            ag1_in = dram.tile([NPB, TCOLS], BF)
            ag1_out = dram.tile([NPAD, TCOLS], BF, addr_space="Shared")
            ag2_in = dram.tile([NPB, TCOLS], BF)
            ag2_out = dram.tile([NPAD, TCOLS], BF)

            def transpose_to(dst_bf, src_bf):
                pst = pss.tile([P, P], BF, tag="tp")
                nc.tensor.transpose(out=pst[:], in_=src_bf, identity=ident[:])
                nc.vector.tensor_copy(out=dst_bf, in_=pst[:])

            # ================= phase 0: LN0 (batched) + a-logits =========
            xt = ep.tile([P, NBLK, D_IN], F32, tag="epA")
            nc.sync.dma_start(
                xt[:], bass.AP(x_own.ap().tensor, 0,
                               [[D_IN, P], [P * D_IN, NBLK], [1, D_IN]]))
            xc = ep.tile([P, NBLK, D_IN], F32, tag="epB")
            mu = wp.tile([P, NBLK], F32, tag="sc0")
            nc.vector.tensor_reduce(out=mu[:], in_=xt[:],
                                    axis=mybir.AxisListType.X, op=OP.add)
            nc.vector.tensor_scalar_mul(out=mu[:], in0=mu[:],
                                        scalar1=1.0 / D_IN)
            nc.vector.tensor_tensor(out=xc[:], in0=xt[:],
                                    in1=_bap(mu[:], [(1, NBLK), (0, D_IN)]),
                                    op=OP.subtract)
            sq = ep.tile([P, NBLK, D_IN], F32, tag="epA")
            nc.vector.tensor_mul(out=sq[:], in0=xc[:], in1=xc[:])
            ss = wp.tile([P, NBLK], F32, tag="sc0")
            nc.vector.tensor_reduce(out=ss[:], in_=sq[:],
                                    axis=mybir.AxisListType.X, op=OP.add)
            sd = wp.tile([P, NBLK], F32, tag="sc1")
            nc.scalar.activation(sd[:], ss[:], AF.Sqrt, bias=eps_t[:],
                                 scale=1.0 / D_IN)
            rstd = wp.tile([P, NBLK], F32, tag="sc2")
            nc.vector.reciprocal(rstd[:], sd[:])
            nc.vector.tensor_tensor(out=xc[:], in0=xc[:],
                                    in1=_bap(rstd[:], [(1, NBLK), (0, D_IN)]),
                                    op=OP.mult)
            nc.vector.tensor_tensor(
                out=xc[:], in0=xc[:],
                in1=_bap(cc[:, CC_GIN:CC_GIN + D_IN], [(0, NBLK), (1, D_IN)]),
                op=OP.mult)
            nc.vector.tensor_tensor(
                out=_bap(tcat[:], [(TCOLS, NBLK), (1, D_IN)]), in0=xc[:],
                in1=_bap(cc[:, CC_BIN:CC_BIN + D_IN], [(0, NBLK), (1, D_IN)]),
                op=OP.add)

            for t in range(NBLK):
                xT = wp.tile([P, P], BF, tag="xT")
                transpose_to(xT[:], tcat[:, t, 0:D_IN])
                ps8_t = pss.tile([P, 130], F32, tag="mms")
                ps8 = ps8_t[:, 0:8]
                nc.tensor.matmul(ps8[:], lhsT=xT[:], rhs=wa8[:],
                                 start=True, stop=True)
                nc.vector.tensor_copy(out=aa_all[:, t, :], in_=ps8[:])

            # als (+NEG on pads) into table cols 128:136 (all blocks at once)
            nc.vector.tensor_tensor(
                out=_bap(tcat[:, :, 128:136].bitcast(F32),
                         [(TCOLS // 2, NBLK), (1, 4)]),
                in0=_bap(aa_all[:], [(8, NBLK), (1, 4)]),
                in1=_bap(afix[:], [(8, NBLK), (1, 4)]), op=OP.add)
            nc.sync.dma_start(
                bass.AP(ag1_in[:].tensor, ag1_in[:].offset,
                        [[TCOLS, P], [P * TCOLS, NBLK], [1, TCOLS]]),
                tcat[:])

            # ================= phase 1: AllGather L1 table ================
            nc.gpsimd.collective_compute(
                "AllGather", OP.bypass, replica_groups=rg,
                ins=[ag1_in[:, :].opt()],
                outs=[ag1_out[:, :].opt()])

            # ================= phase 2: GAT layer 1 =======================
            def ag2_chunk(j):
                r0, r1 = j * NPB // NAG, (j + 1) * NPB // NAG
                nc.gpsimd.collective_compute(
                    "AllGather", OP.bypass, replica_groups=rg,
                    ins=[ag2_in[r0:r1, :].opt()],
                    outs=[ag2_out[j * NPAD // NAG:(j + 1) * NPAD // NAG,
                                  :].opt()])

            GSM = min(24, GMAX)         # small-tile cap; rare bigger blocks
                                        # get a dedicated buffer

            def gather_from(table, idx_sb, l):
                g = ghat[l]
                q = qrr[0] % NSWQ
                qrr[0] += 1
                if g <= GSM:
                    gt = gp.tile([P, GSM, TCOLS], BF, tag="g1s", bufs=4)
                else:
                    gt = gp.tile([P, GMAX, TCOLS], BF, tag="g1b", bufs=1)
                # prepare descriptors, then trigger: the DMA drains
                # off-engine, so several blocks' gathers drain concurrently
                nc.gpsimd.dma_gather(
                    gt[:, 0:g, :], table,
                    idx_sb[:, 8 * int(goff[l]):8 * (int(goff[l]) + g)],
                    P * g, P * g, TCOLS, single_packet=False,
                    prepare_only=True, sem=gsem[q], queue_num=q)
                nc.gpsimd.trigger_dma(count=None, queue_num=q)
                qcnt[q] += 1
                # explicit data-landed gate for the consumers below
                nc.vector.wait_ge(gsem[q], 16 * qcnt[q])
                return gt

            # process groups big-degree-first so the LAST AG2 chunk rides on
            # the cheapest group and the tail exposure shrinks
            procorder = [j * GB + b for j in range(NAG - 1, -1, -1)
                         for b in range(GB)]
            for idx, l in enumerate(procorder):
                g = ghat[l]
                gt = gather_from(ag1_out[:], idx_sb1, l)
                # emit the previously processed group's AG2 chunk here, where
                # its epilogue is long done (never stalls the GpSimd stream)
                if idx % GB == 2 and idx // GB > 0:
                    ag2_chunk(procorder[(idx // GB - 1) * GB] // GB)
                als_v = gt[:, 0:g, 128:136].bitcast(F32)    # [P, g, 4]
                u = wp.tile([P, GMAX, H1], F32, tag="u1")
                nc.vector.tensor_tensor(
                    out=u[:, 0:g, :], in0=als_v,
                    in1=_bap(aa_all[:, l, 4:8], [(0, g), (1, H1)]),
                    op=OP.add)
                nc.vector.scalar_tensor_tensor(
                    out=u[:, 0:g, :], in0=u[:, 0:g, :], scalar=0.2,
                    in1=u[:, 0:g, :], op0=OP.mult, op1=OP.max)
                exf = wp.tile([P, GMAX, H1], F32, tag="ex1")
                nc.scalar.activation(exf[:, 0:g, :], u[:, 0:g, :], AF.Exp)
                nc.vector.tensor_reduce(
                    out=den1[:, l, :], in_=_bap(exf[:], [(1, H1), (H1, g)]),
                    axis=mybir.AxisListType.X, op=OP.add)
                exb = wp.tile([P, GMAX, H1], BF, tag="exb1")
                # flat contiguous cast: tiny-inner-dim copies cost ~300
                # DVE cycles per access-pattern row otherwise
                nc.vector.tensor_copy(out=_bap(exb[:], [(1, g * H1)]),
                                      in_=_bap(exf[:], [(1, g * H1)]))

                psA = ps.tile([P, 512], F32, tag="agg")
                k0 = 0
                while k0 < g:
                    kn = min(KW, g - k0)
                    w4 = wp.tile([P, KW, 512], BF, tag="w4")
                    nc.vector.tensor_tensor(
                        out=_bap(w4[:], [(512, kn), (128, H1), (1, HID)]),
                        in0=_bap(gt[:, k0:k0 + kn, :],
                                 [(TCOLS, kn), (0, H1), (1, HID)]),
                        in1=_bap(exb[:, k0:k0 + kn, :],
                                 [(H1, kn), (1, H1), (0, HID)]),
                        op=OP.mult)
                    for k in range(kn):
                        nc.tensor.matmul(psA[:], lhsT=ident[:],
                                         rhs=w4[:, k, :],
                                         start=(k0 + k == 0),
                                         stop=(k0 + k == g - 1))
                    k0 += kn

                # apply W1 per head: h1 = concat_h(agg_h @ W1_h)
                aggb = wp.tile([P, 512], BF, tag="aggb")
                nc.vector.tensor_copy(out=aggb[:], in_=psA[:])
                pst4 = pss.tile([P, H1, HID], BF, tag="tpx", bufs=1)
                for h in range(H1):
                    nc.tensor.transpose(out=pst4[:, h, :],
                                        in_=aggb[:, h * HID:(h + 1) * HID],
                                        identity=ident[:])
                aggT = wp.tile([P, H1, HID], BF, tag="aggT")
                nc.vector.tensor_copy(out=aggT[:], in_=pst4[:])
                ps1 = ps.tile([P, 512], F32, tag="h1p", bufs=1)
                for h in range(H1):
                    nc.tensor.matmul(ps1[:, h * HID:(h + 1) * HID],
                                     lhsT=aggT[:, h, :],
                                     rhs=w1s[:, h * HID:(h + 1) * HID],
                                     start=True, stop=True)
                nc.vector.tensor_copy(out=h1_all[:, l, :], in_=ps1[:])

                # ---- group epilogue every GB blocks ----
                if idx % GB == GB - 1:
                    j = l // GB
                    s0 = j * GB
                    dv = wp.tile([P, GB, H1], F32, tag="dv1")
                    nc.vector.tensor_scalar_add(
                        out=dv[:], in0=den1[:, s0:s0 + GB, :], scalar1=1e-30)
                    nc.vector.reciprocal(dv[:], dv[:])
                    eA = ep.tile([P, GB, 512], F32, tag="epA")
                    # h1/den (per head) + bias1
                    nc.vector.tensor_tensor(
                        out=_bap(eA[:], [(512, GB), (128, H1), (1, HID)]),
                        in0=_bap(h1_all[:, s0, :],
                                 [(512, GB), (128, H1), (1, HID)]),
                        in1=_bap(dv[:], [(H1, GB), (1, H1), (0, HID)]),
                        op=OP.mult)
                    nc.vector.tensor_tensor(
                        out=eA[:], in0=eA[:],
                        in1=_bap(cc[:, CC_BIAS1:CC_BIAS1 + 512],
                                 [(0, GB), (1, 512)]),
                        op=OP.add)
                    # LN1 batched over the group
                    gmu = wp.tile([P, GB], F32, tag="gsc")
                    nc.vector.tensor_reduce(out=gmu[:], in_=eA[:],
                                            axis=mybir.AxisListType.X,
                                            op=OP.add)
                    nc.vector.tensor_scalar_mul(out=gmu[:], in0=gmu[:],
                                                scalar1=1.0 / 512)
                    nc.vector.tensor_tensor(
                        out=eA[:], in0=eA[:],
                        in1=_bap(gmu[:], [(1, GB), (0, 512)]),
                        op=OP.subtract)
                    eB = ep.tile([P, GB, 512], F32, tag="epB")
                    nc.vector.tensor_tensor(out=eB[:], in0=eA[:], in1=eA[:],
                                            op=OP.mult)
                    gss = wp.tile([P, GB], F32, tag="gsc")
                    nc.vector.tensor_reduce(out=gss[:], in_=eB[:],
                                            axis=mybir.AxisListType.X,
                                            op=OP.add)
                    gsd = wp.tile([P, GB], F32, tag="gsd")
                    nc.scalar.activation(gsd[:], gss[:], AF.Sqrt,
                                         bias=eps_t[:], scale=1.0 / 512)
                    nc.vector.reciprocal(gsd[:], gsd[:])
                    nc.vector.tensor_tensor(
                        out=eA[:], in0=eA[:],
                        in1=_bap(gsd[:], [(1, GB), (0, 512)]), op=OP.mult)
                    nc.vector.tensor_tensor(
                        out=eA[:], in0=eA[:],
                        in1=_bap(cc[:, CC_G1:CC_G1 + 512], [(0, GB), (1, 512)]),
                        op=OP.mult)
                    eG = ep.tile([P, GB, 512], BF, tag="epG")
                    nc.vector.tensor_tensor(
                        out=eG[:], in0=eA[:],
                        in1=_bap(cc[:, CC_B1:CC_B1 + 512], [(0, GB), (1, 512)]),
                        op=OP.add)
                    nc.scalar.activation(eG[:], eG[:], AF.Gelu)

                    # W2 (+fused att2 logits) per block of the group
                    t2 = wp.tile([P, GB, TCOLS], BF, tag="t2")
                    for b in range(GB):
                        ps3_t = pss.tile([P, 130], F32, tag="mms")
                        ps3 = ps3_t[:, 0:130]
                        pstw = pss.tile([P, H1, HID], BF, tag="tpx", bufs=1)
                        for cch in range(4):
                            nc.tensor.transpose(
                                out=pstw[:, cch, :],
                                in_=eG[:, b, cch * P:(cch + 1) * P],
                                identity=ident[:])
                        hT = wp.tile([P, H1, HID], BF, tag="aggT")
                        nc.vector.tensor_copy(out=hT[:], in_=pstw[:])
                        for cch in range(4):
                            nc.tensor.matmul(ps3[:], lhsT=hT[:, cch, :],
                                             rhs=w2s[:, cch, :],
                                             start=(cch == 0), stop=(cch == 3))
                        nc.vector.tensor_copy(out=t2[:, b, 0:128],
                                              in_=ps3[:, 0:128])
                        nc.vector.tensor_tensor(
                            out=t2[:, b, 128:130].bitcast(F32),
                            in0=ps3[:, 128:129], in1=afix[:, s0 + b, 4:5],
                            op=OP.add)
                        nc.vector.tensor_copy(out=ald2[:, s0 + b, :],
                                              in_=ps3[:, 129:130])
                    nc.sync.dma_start(
                        bass.AP(ag2_in[:].tensor,
                                ag2_in[:].offset + s0 * P * TCOLS,
                                [[TCOLS, P], [P * TCOLS, GB], [1, TCOLS]]),
                        t2[:])
                    if idx == NBLK - 1:
                        ag2_chunk(j)

            # ================= phase 4: GAT layer 2 =======================
            for l in range(NBLK):
                g = ghat[l]
                gt = gather_from(ag2_out[:], idx_sb2, l)
                als_v = gt[:, 0:g, 128:130].bitcast(F32)    # [P, g, 1]
                u = wp.tile([P, GMAX, 1], F32, tag="u2")
                nc.vector.tensor_tensor(
                    out=u[:, 0:g, :], in0=als_v,
                    in1=_bap(ald2[:, l, :], [(0, g), (1, 1)]), op=OP.add)
                nc.vector.scalar_tensor_tensor(
                    out=u[:, 0:g, :], in0=u[:, 0:g, :], scalar=0.2,
                    in1=u[:, 0:g, :], op0=OP.mult, op1=OP.max)
                exf = wp.tile([P, GMAX, 1], F32, tag="ex2")
                nc.scalar.activation(exf[:, 0:g, :], u[:, 0:g, :], AF.Exp)
                nc.vector.tensor_reduce(
                    out=den2[:, l, :], in_=_bap(exf[:], [(1, g)]),
                    axis=mybir.AxisListType.X, op=OP.add)
                exb = wp.tile([P, GMAX, 1], BF, tag="exb2")
                nc.vector.tensor_copy(out=_bap(exb[:], [(1, g)]),
                                      in_=_bap(exf[:], [(1, g)]))

                psB_t = ps.tile([P, 512], F32, tag="agg")
                psB = psB_t[:, 0:128]
                k0 = 0
                while k0 < g:
                    kn = min(KW, g - k0)
                    w2m = wp.tile([P, KW, 128], BF, tag="w2m")
                    nc.vector.tensor_tensor(
                        out=w2m[:, 0:kn, :],
                        in0=_bap(gt[:, k0:k0 + kn, :], [(TCOLS, kn), (1, 128)]),
                        in1=_bap(exb[:, k0:k0 + kn, :], [(1, kn), (0, 128)]),
                        op=OP.mult)
                    for k in range(kn):
                        nc.tensor.matmul(psB[:], lhsT=ident[:],
                                         rhs=w2m[:, k, :],
                                         start=(k0 + k == 0),
                                         stop=(k0 + k == g - 1))
                    k0 += kn
                nc.vector.tensor_copy(out=h2_all[:, l, :], in_=psB[:])

            # ---- batched epilogue: 1/den, bias2, LN2, gelu ----
            dv2 = wp.tile([P, NBLK], F32, tag="sc0")
            nc.vector.tensor_scalar_add(
                out=dv2[:], in0=_bap(den2[:], [(1, NBLK)]), scalar1=1e-30)
            nc.vector.reciprocal(dv2[:], dv2[:])
            fA = ep.tile([P, NBLK, 128], F32, tag="epA")
            nc.vector.tensor_tensor(
                out=fA[:], in0=h2_all[:],
                in1=_bap(dv2[:], [(1, NBLK), (0, 128)]), op=OP.mult)
            nc.vector.tensor_tensor(
                out=fA[:], in0=fA[:],
                in1=_bap(cc[:, CC_BIAS2:CC_BIAS2 + 128], [(0, NBLK), (1, 128)]),
                op=OP.add)
            fmu = wp.tile([P, NBLK], F32, tag="sc1")
            nc.vector.tensor_reduce(out=fmu[:], in_=fA[:],
                                    axis=mybir.AxisListType.X, op=OP.add)
            nc.vector.tensor_scalar_mul(out=fmu[:], in0=fmu[:],
                                        scalar1=1.0 / 128)
            nc.vector.tensor_tensor(
                out=fA[:], in0=fA[:], in1=_bap(fmu[:], [(1, NBLK), (0, 128)]),
                op=OP.subtract)
            fB = ep.tile([P, NBLK, 128], F32, tag="epB")
            nc.vector.tensor_tensor(out=fB[:], in0=fA[:], in1=fA[:],
                                    op=OP.mult)
            fss = wp.tile([P, NBLK], F32, tag="sc2")
            nc.vector.tensor_reduce(out=fss[:], in_=fB[:],
                                    axis=mybir.AxisListType.X, op=OP.add)
            fsd = wp.tile([P, NBLK], F32, tag="sc3")
            nc.scalar.activation(fsd[:], fss[:], AF.Sqrt, bias=eps_t[:],
                                 scale=1.0 / 128)
            nc.vector.reciprocal(fsd[:], fsd[:])
            nc.vector.tensor_tensor(
                out=fA[:], in0=fA[:], in1=_bap(fsd[:], [(1, NBLK), (0, 128)]),
                op=OP.mult)
            nc.vector.tensor_tensor(
                out=fA[:], in0=fA[:],
                in1=_bap(cc[:, CC_G2:CC_G2 + 128], [(0, NBLK), (1, 128)]),
                op=OP.mult)
            fG = ep.tile([P, NBLK, 128], BF, tag="epG")
            nc.vector.tensor_tensor(
                out=fG[:], in0=fA[:],
                in1=_bap(cc[:, CC_B2:CC_B2 + 128], [(0, NBLK), (1, 128)]),
                op=OP.add)
            nc.scalar.activation(fG[:], fG[:], AF.Gelu)

            # ---- output head + batched log_softmax ----
            for l in range(NBLK):
                hoT = wp.tile([P, P], BF, tag="xT")
                transpose_to(hoT[:], fG[:, l, :])
                pso_t = pss.tile([P, 130], F32, tag="mms")
                pso = pso_t[:, 0:D_OUT]
                nc.tensor.matmul(pso[:], lhsT=hoT[:], rhs=wos[:],
                                 start=True, stop=True)
                nc.vector.tensor_copy(out=z_all[:, l, :], in_=pso[:])

            nc.vector.tensor_tensor(
                out=z_all[:], in0=z_all[:],
                in1=_bap(cc[:, CC_BO:CC_BO + D_OUT], [(0, NBLK), (1, D_OUT)]),
                op=OP.add)
            zm = wp.tile([P, NBLK], F32, tag="sc0")
            nc.vector.tensor_reduce(out=zm[:], in_=z_all[:],
                                    axis=mybir.AxisListType.X, op=OP.max)
            nc.vector.tensor_tensor(
                out=z_all[:], in0=z_all[:],
                in1=_bap(zm[:], [(1, NBLK), (0, D_OUT)]), op=OP.subtract)
            ez = ep.tile([P, NBLK, D_OUT], F32, tag="epB")
            nc.scalar.activation(ez[:], z_all[:], AF.Exp)
            sden = wp.tile([P, NBLK], F32, tag="sc1")
            nc.vector.tensor_reduce(out=sden[:], in_=ez[:],
                                    axis=mybir.AxisListType.X, op=OP.add)
            lnd = wp.tile([P, NBLK], F32, tag="sc2")
            nc.scalar.activation(lnd[:], sden[:], AF.Ln)
            nc.vector.tensor_tensor(
                out=z_all[:], in0=z_all[:],
                in1=_bap(lnd[:], [(1, NBLK), (0, D_OUT)]), op=OP.subtract)
            nc.sync.dma_start(
                bass.AP(out.ap().tensor, 0,
                        [[D_OUT, P], [P * D_OUT, NBLK], [1, D_OUT]]),
                z_all[:])

    nc.compile()
    return nc


_CACHE = {}
_LAST_RUN = {}


def kernel(x, edge_index, g_in, b_in, W1, att1_s, att1_d, bias1, g1, b1,
           W2, att2_s, att2_d, bias2, g2, b2, Wo, bo):
    prep = prepare_inputs(x, edge_index)
    wts = prepare_weights(W1, att1_s, att1_d, bias1, g1, b1, g_in, b_in,
                          W2, att2_s, att2_d, bias2, g2, b2, Wo, bo)

    key = tuple(prep["ghat"])
    if key not in _CACHE:
        _CACHE[key] = build_program(prep["ghat"])
    nc = _CACHE[key]

    in_maps = []
    for c in range(NCORES):
        in_maps.append({
            "x_own": prep["x_own"][c],
            "idxw1": prep["idxw1"][c],
            "idxw2": prep["idxw2"][c],
            "alsfix": prep["alsfix"][c],
            "w1a8": wts["w1a8"],
            "w1f": wts["w1f"],
            "w2ext": wts["w2ext"].astype(BF16),
            "woext": wts["woext"],
            "colconst": wts["colconst"],
        })

    _LAST_RUN.update(nc=nc, in_maps=in_maps, prep=prep)
    res = bass_utils.run_bass_kernel_spmd(nc, in_maps,
                                          core_ids=list(range(NCORES)))
    outs = [res.results[c]["out"] for c in range(NCORES)]

    newid = prep["newid"]
    blk = newid // P
    core = blk % NCORES
    row = (blk // NCORES) * P + newid % P
    full = np.empty((N, D_OUT), dtype=np.float32)
    for c in range(NCORES):
        sel = core == c
        full[sel] = outs[c][row[sel]]
    return full


# revision 43
# speedup vs baseline: 1.0274x; 1.0274x over previous
"""Distributed 2-layer GAT kernel for 8 Trainium2 NeuronCores (v2).

Strategy (host graph preprocessing + device SPMD kernel):
  * Nodes are relabeled by in-degree (ascending) and padded to 20480 ids.
    Blocks of 128 consecutive ids have near-uniform in-degree; the 160
    blocks are dealt round-robin to 8 cores, so every core runs the same
    per-block degree schedule ghat[l] (compile-time constant SPMD program).
  * Layer-1 table row (256 bf16 cols, 512B) = [LN(x) 128 | a_src logits
    4xf32 | pad]. W1 is applied AFTER aggregation (linearity of the GAT
    sum), which shrinks the AllGather 2.5x and the gather rows 2.5x vs
    shipping x@W1.
  * Layer-2 table row = [h@W2 128 | a_src logit f32 | pad].
  * Edge slots are dst-major; one dma_gather per 128-dst block pulls all
    in-edge rows; softmax + weighted sum via PSUM-accumulated identity
    matmuls; epilogues (1/den, bias, LN, gelu, log_softmax) are batched
    across blocks to minimize DVE instruction count.
  * AllGather outputs are addr_space="Shared" (HBM-pair buffers).
  * Pad edge slots point to table row 0 with a_src forced to -1e9 on the
    host (alsfix), making exp(leaky_relu(...)) == 0 exactly.
"""
import sys

sys.path.insert(0, "/opt/trn_rl_repo")

import numpy as np
import ml_dtypes

from concourse import bass, bacc, tile, mybir
from concourse import bass_utils
from concourse.masks import make_identity

BF16 = ml_dtypes.bfloat16
F32 = mybir.dt.float32
BF = mybir.dt.bfloat16
I16 = mybir.dt.int16
AF = mybir.ActivationFunctionType
OP = mybir.AluOpType

# problem constants
N, E = 20000, 320000
D_IN, HID, D_OUT = 128, 128, 32
H1, H2 = 4, 1
EPS = 1e-5

NCORES = 8
P = 128
NPAD = 20480            # padded node count: 160 blocks of 128
NBLK_G = NPAD // P      # 160 global blocks
NPB = NPAD // NCORES    # 2560 nodes per core
NBLK = NPB // P         # 20 blocks per core
NEG = -1e9

TCOLS = 256             # both tables: 256 bf16 cols = 512B rows
KW = 12                 # weight-multiply chunk (SBUF bound)
NAG = 4                 # AllGather chunks
GB = NBLK // NAG        # blocks per phase-2 epilogue group (5)
NSWQ = 4                # SWDGE queues

# colconst column layout (f32, replicated on all 128 partitions)
CC_GIN, CC_BIN = 0, 128
CC_G1, CC_B1, CC_BIAS1 = 256, 768, 1280
CC_G2, CC_B2, CC_BIAS2 = 1792, 1920, 2048
CC_BO = 2176
NCC = 2208


def _tid(n):
    """table row id of padded-node id n: AllGather chunk j holds blocks
    [5j, 5j+5) of every core, rank-major within the chunk."""
    blk = n // P
    c = blk % NCORES
    l = blk // NCORES
    cb = NBLK // NAG                     # blocks per AG chunk
    return ((l // cb) * (NPAD // NAG) + c * (NPB // NAG)
            + (l % cb) * P + n % P)


def prepare_inputs(x, edge_index):
    """Host graph preprocessing -> per-core arrays + degree schedule."""
    x = np.asarray(x, dtype=np.float32)
    ei = np.asarray(edge_index)
    src = np.concatenate([ei[0], np.arange(N, dtype=ei.dtype)]).astype(np.int64)
    dst = np.concatenate([ei[1], np.arange(N, dtype=ei.dtype)]).astype(np.int64)

    deg = np.bincount(dst, minlength=N)
    order = np.argsort(deg, kind="stable")        # orig node ids, deg ascending
    newid = np.empty(N, dtype=np.int64)           # orig -> padded id
    newid[order] = np.arange(N) + (NPAD - N)      # pads occupy ids [0, 480)

    degp = np.zeros(NPAD, dtype=np.int64)
    degp[newid] = deg
    gmax = degp.reshape(NBLK_G, P).max(axis=1)
    ghat = gmax.reshape(NBLK, NCORES).max(axis=1)         # per local block idx
    S = int(P * ghat.sum())                                # slots per core

    # CSR of in-edges keyed by new dst id
    nd = newid[dst]
    csr_order = np.argsort(nd, kind="stable")
    nsrc_sorted = newid[src[csr_order]]
    indptr = np.zeros(NPAD + 1, dtype=np.int64)
    np.cumsum(np.bincount(nd, minlength=NPAD), out=indptr[1:])

    # table-1 layout: single-shot AllGather output is rank-major
    blkg = np.arange(NPAD) // P
    tid1_of = (blkg % NCORES) * NPB + (blkg // NCORES) * P + np.arange(NPAD) % P
    # table-2 layout: chunked AllGather (chunk-major, rank-major within chunk)
    tid2_of = _tid(np.arange(NPAD))

    goff = np.zeros(NBLK, dtype=np.int64)                  # k-slot offsets
    goff[1:] = np.cumsum(ghat)[:-1]

    idxw1 = np.zeros((NCORES, P, S // 16), dtype=np.int16)
    idxw2 = np.zeros((NCORES, P, S // 16), dtype=np.int16)
    x_own = np.zeros((NCORES, NPB, D_IN), dtype=np.float32)
    alsfix = np.zeros((NCORES, NPB, 8), dtype=np.float32)

    inv_new = np.full(NPAD, -1, dtype=np.int64)
    inv_new[newid] = np.arange(N)

    for c in range(NCORES):
        gblk = np.arange(NBLK) * NCORES + c                # global block ids
        nid = (gblk[:, None] * P + np.arange(P)).reshape(-1)   # [NPB] padded id
        ov = inv_new[nid]                                  # orig node or -1
        real = ov >= 0
        x_own[c][real] = x[ov[real]]
        alsfix[c][~real, :] = NEG

        src_flat = np.zeros(S, dtype=np.int64)             # dummy -> node 0
        for l in range(NBLK):
            d0 = nid[l * P:(l + 1) * P]                    # padded ids of block
            base = goff[l] * P
            for p in range(P):
                d = d0[p]
                s0, s1 = indptr[d], indptr[d + 1]
                ks = np.arange(s1 - s0)
                src_flat[base + ks * P + p] = nsrc_sorted[s0:s1]
        for idxw, tid_of in ((idxw1, tid1_of), (idxw2, tid2_of)):
            idx_flat = tid_of[src_flat].astype(np.int16)
            idxw[c] = np.tile(idx_flat.reshape(S // 16, 16).T, (NCORES, 1))

    return {
        "ghat": [int(g) for g in ghat],
        "S": S,
        "idxw1": idxw1,
        "idxw2": idxw2,
        "x_own": x_own,
        "alsfix": alsfix,
        "newid": newid,
    }


def prepare_weights(W1, att1_s, att1_d, bias1, g1, b1, g_in, b_in,
                    W2, att2_s, att2_d, bias2, g2, b2, Wo, bo):
    W1 = np.asarray(W1, np.float32)
    W2 = np.asarray(W2, np.float32)
    W1h = W1.reshape(D_IN, H1, HID)
    w1a8 = np.zeros((D_IN, 8), dtype=BF16)
    w1a8[:, 0:4] = np.einsum("khc,hc->kh", W1h, np.asarray(att1_s, np.float32))
    w1a8[:, 4:8] = np.einsum("khc,hc->kh", W1h, np.asarray(att1_d, np.float32))
    w1f = W1.astype(BF16)                                  # [128, 512]

    w2e = np.zeros((4 * HID, 130), dtype=np.float32)
    w2e[:, :128] = W2
    w2e[:, 128] = W2 @ np.asarray(att2_s, np.float32)[0]
    w2e[:, 129] = W2 @ np.asarray(att2_d, np.float32)[0]
    # pack [512, 130] -> [128, 4, 130] (partition p holds rows p, 128+p, ...)
    w2ext = np.ascontiguousarray(
        w2e.reshape(4, P, 130).transpose(1, 0, 2)).astype(BF16)

    woext = np.asarray(Wo, np.float32).astype(BF16)

    cc = np.zeros(NCC, dtype=np.float32)
    cc[CC_GIN:CC_GIN + 128] = g_in
    cc[CC_BIN:CC_BIN + 128] = b_in
    cc[CC_G1:CC_G1 + 512] = g1
    cc[CC_B1:CC_B1 + 512] = b1
    cc[CC_BIAS1:CC_BIAS1 + 512] = bias1
    cc[CC_G2:CC_G2 + 128] = g2
    cc[CC_B2:CC_B2 + 128] = b2
    cc[CC_BIAS2:CC_BIAS2 + 128] = bias2
    cc[CC_BO:CC_BO + 32] = bo
    colconst = np.tile(cc[None, :], (P, 1))

    return {"w1a8": w1a8, "w1f": w1f, "w2ext": w2ext.reshape(P, 4 * 130),
            "woext": woext, "colconst": colconst}


def _bap(ap, dims):
    """AP with explicit free-dim [step, count] pairs (partition dim kept)."""
    return bass.AP(ap.tensor, ap.offset, [ap.ap[0]] + [list(d) for d in dims])


def build_program(ghat, num_devices=NCORES):
    GMAX = int(max(ghat))
    goff = np.zeros(NBLK, dtype=np.int64)
    goff[1:] = np.cumsum(ghat)[:-1]
    S = int(P * sum(ghat))

    nc = bacc.Bacc("TRN2", target_bir_lowering=False, debug=False,
                   num_devices=num_devices, num_swdge_queues=NSWQ)

    x_own = nc.dram_tensor("x_own", [NPB, D_IN], F32, kind="ExternalInput")
    idxw1 = nc.dram_tensor("idxw1", [P, S // 16], I16, kind="ExternalInput")
    idxw2 = nc.dram_tensor("idxw2", [P, S // 16], I16, kind="ExternalInput")
    alsfix = nc.dram_tensor("alsfix", [NPB, 8], F32, kind="ExternalInput")
    w1a8 = nc.dram_tensor("w1a8", [D_IN, 8], BF, kind="ExternalInput")
    w1f = nc.dram_tensor("w1f", [D_IN, 512], BF, kind="ExternalInput")
    w2ext = nc.dram_tensor("w2ext", [P, 4 * 130], BF, kind="ExternalInput")
    woext = nc.dram_tensor("woext", [P, D_OUT], BF, kind="ExternalInput")
    colconst = nc.dram_tensor("colconst", [P, NCC], F32, kind="ExternalInput")
    out = nc.dram_tensor("out", [NPB, D_OUT], F32, kind="ExternalOutput")

    rg = [list(range(num_devices))]
    qrr = [0]
    gsem = [nc.alloc_semaphore(f"gsem{q}") for q in range(NSWQ)]
    qcnt = [0] * NSWQ

    with tile.TileContext(nc) as tc:
        with (
            tc.tile_pool(name="cst", bufs=1) as cst,
            tc.tile_pool(name="wp", bufs=2) as wp,
            tc.tile_pool(name="ep", bufs=1) as ep,
            tc.tile_pool(name="gp", bufs=3) as gp,
            tc.tile_pool(name="ps", bufs=2, space="PSUM") as ps,
            tc.tile_pool(name="pss", bufs=2, space="PSUM") as pss,
            tc.tile_pool(name="dram", bufs=1, space="DRAM") as dram,
        ):
            # ---- constants ----
            for q in range(NSWQ):
                nc.gpsimd.sem_clear(gsem[q])
            ident = cst.tile([P, P], BF)
            make_identity(nc, ident[:])
            wa8 = cst.tile([P, 8], BF)
            nc.sync.dma_start(wa8[:], w1a8[:])
            w1s = cst.tile([P, 512], BF)
            nc.sync.dma_start(w1s[:], w1f[:])
            w2s = cst.tile([P, 4, 130], BF)
            nc.sync.dma_start(w2s[:], w2ext[:])
            wos = cst.tile([P, D_OUT], BF)
            nc.sync.dma_start(wos[:], woext[:])
            cc = cst.tile([P, NCC], F32)
            nc.sync.dma_start(cc[:], colconst[:])
            idx_sb1 = cst.tile([P, S // 16], I16)
            nc.sync.dma_start(idx_sb1[:], idxw1[:])
            idx_sb2 = cst.tile([P, S // 16], I16)
            nc.sync.dma_start(idx_sb2[:], idxw2[:])
            afix = cst.tile([P, NBLK, 8], F32)
            nc.sync.dma_start(
                afix[:], bass.AP(alsfix.ap().tensor, 0,
                                 [[8, P], [8 * P, NBLK], [1, 8]]))
            eps_t = cst.tile([P, 1], F32)
            nc.vector.memset(eps_t[:], EPS)

            # persistent state
            aa_all = cst.tile([P, NBLK, 8], F32)    # als(4) | ald(4) per block
            den1 = cst.tile([P, NBLK, H1], F32)
            ald2 = cst.tile([P, NBLK, 1], F32)
            den2 = cst.tile([P, NBLK, 1], F32)
            h1_all = cst.tile([P, NBLK, 512], BF)   # pre-epilogue h1 (20KB)
            h2_all = cst.tile([P, NBLK, 128], F32)  # pre-epilogue h2 (10KB)
            z_all = cst.tile([P, NBLK, D_OUT], F32)
            tcat = cst.tile([P, NBLK, TCOLS], BF)   # L1 table staging (10KB)

            ag1_in = dram.tile([NPB, TCOLS], BF)
            ag1_out = dram.tile([NPAD, TCOLS], BF, addr_space="Shared")
            ag2_in = dram.tile([NPB, TCOLS], BF)
            ag2_out = dram.tile([NPAD, TCOLS], BF)

            def transpose_to(dst_bf, src_bf):
                pst = pss.tile([P, P], BF, tag="tp")
                nc.tensor.transpose(out=pst[:], in_=src_bf, identity=ident[:])
                nc.vector.tensor_copy(out=dst_bf, in_=pst[:])

            # ================= phase 0: LN0 (batched) + a-logits =========
            xt = ep.tile([P, NBLK, D_IN], F32, tag="epA")
            nc.sync.dma_start(
                xt[:], bass.AP(x_own.ap().tensor, 0,
                               [[D_IN, P], [P * D_IN, NBLK], [1, D_IN]]))
            xc = ep.tile([P, NBLK, D_IN], F32, tag="epB")
            mu = wp.tile([P, NBLK], F32, tag="sc0")
            nc.vector.tensor_reduce(out=mu[:], in_=xt[:],
                                    axis=mybir.AxisListType.X, op=OP.add)
            nc.vector.tensor_scalar_mul(out=mu[:], in0=mu[:],
                                        scalar1=1.0 / D_IN)
            nc.vector.tensor_tensor(out=xc[:], in0=xt[:],
                                    in1=_bap(mu[:], [(1, NBLK), (0, D_IN)]),
                                    op=OP.subtract)
            sq = ep.tile([P, NBLK, D_IN], F32, tag="epA")
            nc.vector.tensor_mul(out=sq[:], in0=xc[:], in1=xc[:])
            ss = wp.tile([P, NBLK], F32, tag="sc0")
            nc.vector.tensor_reduce(out=ss[:], in_=sq[:],
                                    axis=mybir.AxisListType.X, op=OP.add)
            sd = wp.tile([P, NBLK], F32, tag="sc1")
            nc.scalar.activation(sd[:], ss[:], AF.Sqrt, bias=eps_t[:],
                                 scale=1.0 / D_IN)
            rstd = wp.tile([P, NBLK], F32, tag="sc2")
            nc.vector.reciprocal(rstd[:], sd[:])
            nc.vector.tensor_tensor(out=xc[:], in0=xc[:],
                                    in1=_bap(rstd[:], [(1, NBLK), (0, D_IN)]),
                                    op=OP.mult)
            nc.vector.tensor_tensor(
                out=xc[:], in0=xc[:],
                in1=_bap(cc[:, CC_GIN:CC_GIN + D_IN], [(0, NBLK), (1, D_IN)]),
                op=OP.mult)
            nc.vector.tensor_tensor(
                out=_bap(tcat[:], [(TCOLS, NBLK), (1, D_IN)]), in0=xc[:],
                in1=_bap(cc[:, CC_BIN:CC_BIN + D_IN], [(0, NBLK), (1, D_IN)]),
                op=OP.add)

            for t in range(NBLK):
                xT = wp.tile([P, P], BF, tag="xT")
                transpose_to(xT[:], tcat[:, t, 0:D_IN])
                ps8_t = pss.tile([P, 130], F32, tag="mms")
                ps8 = ps8_t[:, 0:8]
                nc.tensor.matmul(ps8[:], lhsT=xT[:], rhs=wa8[:],
                                 start=True, stop=True)
                nc.vector.tensor_copy(out=aa_all[:, t, :], in_=ps8[:])

            # als (+NEG on pads) into table cols 128:136 (all blocks at once)
            nc.vector.tensor_tensor(
                out=_bap(tcat[:, :, 128:136].bitcast(F32),
                         [(TCOLS // 2, NBLK), (1, 4)]),
                in0=_bap(aa_all[:], [(8, NBLK), (1, 4)]),
                in1=_bap(afix[:], [(8, NBLK), (1, 4)]), op=OP.add)
            nc.sync.dma_start(
                bass.AP(ag1_in[:].tensor, ag1_in[:].offset,
                        [[TCOLS, P], [P * TCOLS, NBLK], [1, TCOLS]]),
                tcat[:])

            # ================= phase 1: AllGather L1 table ================
            nc.gpsimd.collective_compute(
                "AllGather", OP.bypass, replica_groups=rg,
                ins=[ag1_in[:, :].opt()],
                outs=[ag1_out[:, :].opt()])

            # ================= phase 2: GAT layer 1 =======================
            def ag2_chunk(j):
                r0, r1 = j * NPB // NAG, (j + 1) * NPB // NAG
                nc.gpsimd.collective_compute(
                    "AllGather", OP.bypass, replica_groups=rg,
                    ins=[ag2_in[r0:r1, :].opt()],
                    outs=[ag2_out[j * NPAD // NAG:(j + 1) * NPAD // NAG,
                                  :].opt()])

            GSM = min(24, GMAX)         # small-tile cap; rare bigger blocks
                                        # get a dedicated buffer

            def gather_from(table, idx_sb, l):
                g = ghat[l]
                q = qrr[0] % NSWQ
                qrr[0] += 1
                if g <= GSM:
                    gt = gp.tile([P, GSM, TCOLS], BF, tag="g1s", bufs=4)
                else:
                    gt = gp.tile([P, GMAX, TCOLS], BF, tag="g1b", bufs=1)
                # prepare descriptors, then trigger: the DMA drains
                # off-engine, so several blocks' gathers drain concurrently
                nc.gpsimd.dma_gather(
                    gt[:, 0:g, :], table,
                    idx_sb[:, 8 * int(goff[l]):8 * (int(goff[l]) + g)],
                    P * g, P * g, TCOLS, single_packet=False,
                    prepare_only=True, sem=gsem[q], queue_num=q)
                nc.gpsimd.trigger_dma(count=None, queue_num=q)
                qcnt[q] += 1
                # explicit data-landed gate for the consumers below
                nc.vector.wait_ge(gsem[q], 16 * qcnt[q])
                return gt

            # process groups big-degree-first so the LAST AG2 chunk rides on
            # the cheapest group and the tail exposure shrinks
            procorder = [j * GB + b for j in range(NAG - 1, -1, -1)
                         for b in range(GB)]
            for idx, l in enumerate(procorder):
                g = ghat[l]
                gt = gather_from(ag1_out[:], idx_sb1, l)
                # emit the previously processed group's AG2 chunk here, where
                # its epilogue is long done (never stalls the GpSimd stream)
                if idx % GB == 2 and idx // GB > 0:
                    ag2_chunk(procorder[(idx // GB - 1) * GB] // GB)
                als_v = gt[:, 0:g, 128:136].bitcast(F32)    # [P, g, 4]
                u = wp.tile([P, GMAX, H1], F32, tag="u1")
                nc.vector.tensor_tensor(
                    out=u[:, 0:g, :], in0=als_v,
                    in1=_bap(aa_all[:, l, 4:8], [(0, g), (1, H1)]),
                    op=OP.add)
                nc.vector.scalar_tensor_tensor(
                    out=u[:, 0:g, :], in0=u[:, 0:g, :], scalar=0.2,
                    in1=u[:, 0:g, :], op0=OP.mult, op1=OP.max)
                exf = wp.tile([P, GMAX, H1], F32, tag="ex1")
                nc.scalar.activation(exf[:, 0:g, :], u[:, 0:g, :], AF.Exp)
                nc.vector.tensor_reduce(
                    out=den1[:, l, :], in_=_bap(exf[:], [(1, H1), (H1, g)]),
                    axis=mybir.AxisListType.X, op=OP.add)
                exb = wp.tile([P, GMAX, H1], BF, tag="exb1")
                # flat contiguous cast: tiny-inner-dim copies cost ~300
                # DVE cycles per access-pattern row otherwise
                nc.vector.tensor_copy(out=_bap(exb[:], [(1, g * H1)]),
                                      in_=_bap(exf[:], [(1, g * H1)]))

                psA = ps.tile([P, 512], F32, tag="agg")
                k0 = 0
                while k0 < g:
                    kn = min(KW, g - k0)
                    w4 = wp.tile([P, KW, 512], BF, tag="w4")
                    nc.vector.tensor_tensor(
                        out=_bap(w4[:], [(512, kn), (128, H1), (1, HID)]),
                        in0=_bap(gt[:, k0:k0 + kn, :],
                                 [(TCOLS, kn), (0, H1), (1, HID)]),
                        in1=_bap(exb[:, k0:k0 + kn, :],
                                 [(H1, kn), (1, H1), (0, HID)]),
                        op=OP.mult)
                    for k in range(kn):
                        nc.tensor.matmul(psA[:], lhsT=ident[:],
                                         rhs=w4[:, k, :],
                                         start=(k0 + k == 0),
                                         stop=(k0 + k == g - 1))
                    k0 += kn

                # apply W1 per head: h1 = concat_h(agg_h @ W1_h)
                aggb = wp.tile([P, 512], BF, tag="aggb")
                nc.vector.tensor_copy(out=aggb[:], in_=psA[:])
                pst4 = pss.tile([P, H1, HID], BF, tag="tpx", bufs=1)
                for h in range(H1):
                    nc.tensor.transpose(out=pst4[:, h, :],
                                        in_=aggb[:, h * HID:(h + 1) * HID],
                                        identity=ident[:])
                aggT = wp.tile([P, H1, HID], BF, tag="aggT")
                nc.vector.tensor_copy(out=aggT[:], in_=pst4[:])
                ps1 = ps.tile([P, 512], F32, tag="h1p", bufs=1)
                for h in range(H1):
                    nc.tensor.matmul(ps1[:, h * HID:(h + 1) * HID],
                                     lhsT=aggT[:, h, :],
                                     rhs=w1s[:, h * HID:(h + 1) * HID],
                                     start=True, stop=True)
                nc.vector.tensor_copy(out=h1_all[:, l, :], in_=ps1[:])

                # ---- group epilogue every GB blocks ----
                if idx % GB == GB - 1:
                    j = l // GB
                    s0 = j * GB
                    dv = wp.tile([P, GB, H1], F32, tag="dv1")
                    nc.vector.tensor_scalar_add(
                        out=dv[:], in0=den1[:, s0:s0 + GB, :], scalar1=1e-30)
                    nc.vector.reciprocal(dv[:], dv[:])
                    eA = ep.tile([P, GB, 512], F32, tag="epA")
                    # h1/den (per head) + bias1
                    nc.vector.tensor_tensor(
                        out=_bap(eA[:], [(512, GB), (128, H1), (1, HID)]),
                        in0=_bap(h1_all[:, s0, :],
                                 [(512, GB), (128, H1), (1, HID)]),
                        in1=_bap(dv[:], [(H1, GB), (1, H1), (0, HID)]),
                        op=OP.mult)
                    nc.vector.tensor_tensor(
                        out=eA[:], in0=eA[:],
                        in1=_bap(cc[:, CC_BIAS1:CC_BIAS1 + 512],
                                 [(0, GB), (1, 512)]),
                        op=OP.add)
                    # LN1 batched over the group
                    gmu = wp.tile([P, GB], F32, tag="gsc")
                    nc.vector.tensor_reduce(out=gmu[:], in_=eA[:],
                                            axis=mybir.AxisListType.X,
                                            op=OP.add)
                    nc.vector.tensor_scalar_mul(out=gmu[:], in0=gmu[:],
                                                scalar1=1.0 / 512)
                    nc.vector.tensor_tensor(
                        out=eA[:], in0=eA[:],
                        in1=_bap(gmu[:], [(1, GB), (0, 512)]),
                        op=OP.subtract)
                    eB = ep.tile([P, GB, 512], F32, tag="epB")
                    nc.vector.tensor_tensor(out=eB[:], in0=eA[:], in1=eA[:],
                                            op=OP.mult)
                    gss = wp.tile([P, GB], F32, tag="gsc")
                    nc.vector.tensor_reduce(out=gss[:], in_=eB[:],
                                            axis=mybir.AxisListType.X,
                                            op=OP.add)
                    gsd = wp.tile([P, GB], F32, tag="gsd")
                    nc.scalar.activation(gsd[:], gss[:], AF.Sqrt,
                                         bias=eps_t[:], scale=1.0 / 512)
                    nc.vector.reciprocal(gsd[:], gsd[:])
                    nc.vector.tensor_tensor(
                        out=eA[:], in0=eA[:],
                        in1=_bap(gsd[:], [(1, GB), (0, 512)]), op=OP.mult)
                    nc.vector.tensor_tensor(
                        out=eA[:], in0=eA[:],
                        in1=_bap(cc[:, CC_G1:CC_G1 + 512], [(0, GB), (1, 512)]),
                        op=OP.mult)
                    eG = ep.tile([P, GB, 512], BF, tag="epG")
                    nc.vector.tensor_tensor(
                        out=eG[:], in0=eA[:],
                        in1=_bap(cc[:, CC_B1:CC_B1 + 512], [(0, GB), (1, 512)]),
                        op=OP.add)
                    nc.scalar.activation(eG[:], eG[:], AF.Gelu)

                    # W2 (+fused att2 logits) per block of the group
                    t2 = wp.tile([P, GB, TCOLS], BF, tag="t2")
                    for b in range(GB):
                        ps3_t = pss.tile([P, 130], F32, tag="mms")
                        ps3 = ps3_t[:, 0:130]
                        pstw = pss.tile([P, H1, HID], BF, tag="tpx", bufs=1)
                        for cch in range(4):
                            nc.tensor.transpose(
                                out=pstw[:, cch, :],
                                in_=eG[:, b, cch * P:(cch + 1) * P],
                                identity=ident[:])
                        hT = wp.tile([P, H1, HID], BF, tag="aggT")
                        nc.vector.tensor_copy(out=hT[:], in_=pstw[:])
                        for cch in range(4):
                            nc.tensor.matmul(ps3[:], lhsT=hT[:, cch, :],
                                             rhs=w2s[:, cch, :],
                                             start=(cch == 0), stop=(cch == 3))
                        nc.vector.tensor_copy(out=t2[:, b, 0:128],
                                              in_=ps3[:, 0:128])
                        nc.vector.tensor_tensor(
                            out=t2[:, b, 128:130].bitcast(F32),
                            in0=ps3[:, 128:129], in1=afix[:, s0 + b, 4:5],
                            op=OP.add)
                        nc.vector.tensor_copy(out=ald2[:, s0 + b, :],
                                              in_=ps3[:, 129:130])
                    nc.sync.dma_start(
                        bass.AP(ag2_in[:].tensor,
                                ag2_in[:].offset + s0 * P * TCOLS,
                                [[TCOLS, P], [P * TCOLS, GB], [1, TCOLS]]),
                        t2[:])
                    if idx == NBLK - 1:
                        ag2_chunk(j)

            # ================= phase 4: GAT layer 2 =======================
            for l in range(NBLK):
                g = ghat[l]
                gt = gather_from(ag2_out[:], idx_sb2, l)
                als_v = gt[:, 0:g, 128:130].bitcast(F32)    # [P, g, 1]
                u = wp.tile([P, GMAX, 1], F32, tag="u2")
                nc.vector.tensor_tensor(
                    out=u[:, 0:g, :], in0=als_v,
                    in1=_bap(ald2[:, l, :], [(0, g), (1, 1)]), op=OP.add)
                nc.vector.scalar_tensor_tensor(
                    out=u[:, 0:g, :], in0=u[:, 0:g, :], scalar=0.2,
                    in1=u[:, 0:g, :], op0=OP.mult, op1=OP.max)
                exf = wp.tile([P, GMAX, 1], F32, tag="ex2")
                nc.scalar.activation(exf[:, 0:g, :], u[:, 0:g, :], AF.Exp)
                nc.vector.tensor_reduce(
                    out=den2[:, l, :], in_=_bap(exf[:], [(1, g)]),
                    axis=mybir.AxisListType.X, op=OP.add)
                exb = wp.tile([P, GMAX, 1], BF, tag="exb2")
                nc.vector.tensor_copy(out=_bap(exb[:], [(1, g)]),
                                      in_=_bap(exf[:], [(1, g)]))

                psB_t = ps.tile([P, 512], F32, tag="agg")
                psB = psB_t[:, 0:128]
                k0 = 0
                while k0 < g:
                    kn = min(KW, g - k0)
                    w2m = wp.tile([P, KW, 128], BF, tag="w2m")
                    nc.vector.tensor_tensor(
                        out=w2m[:, 0:kn, :],
                        in0=_bap(gt[:, k0:k0 + kn, :], [(TCOLS, kn), (1, 128)]),
                        in1=_bap(exb[:, k0:k0 + kn, :], [(1, kn), (0, 128)]),
                        op=OP.mult)
                    for k in range(kn):
                        nc.tensor.matmul(psB[:], lhsT=ident[:],
                                         rhs=w2m[:, k, :],
                                         start=(k0 + k == 0),
                                         stop=(k0 + k == g - 1))
                    k0 += kn
                nc.vector.tensor_copy(out=h2_all[:, l, :], in_=psB[:])

            # ---- batched epilogue: 1/den, bias2, LN2, gelu ----
            dv2 = wp.tile([P, NBLK], F32, tag="sc0")
            nc.vector.tensor_scalar_add(
                out=dv2[:], in0=_bap(den2[:], [(1, NBLK)]), scalar1=1e-30)
            nc.vector.reciprocal(dv2[:], dv2[:])
            fA = ep.tile([P, NBLK, 128], F32, tag="epA")
            nc.vector.tensor_tensor(
                out=fA[:], in0=h2_all[:],
                in1=_bap(dv2[:], [(1, NBLK), (0, 128)]), op=OP.mult)
            nc.vector.tensor_tensor(
                out=fA[:], in0=fA[:],
                in1=_bap(cc[:, CC_BIAS2:CC_BIAS2 + 128], [(0, NBLK), (1, 128)]),
                op=OP.add)
            fmu = wp.tile([P, NBLK], F32, tag="sc1")
            nc.vector.tensor_reduce(out=fmu[:], in_=fA[:],
                                    axis=mybir.AxisListType.X, op=OP.add)
            nc.vector.tensor_scalar_mul(out=fmu[:], in0=fmu[:],
                                        scalar1=1.0 / 128)
            nc.vector.tensor_tensor(
                out=fA[:], in0=fA[:], in1=_bap(fmu[:], [(1, NBLK), (0, 128)]),
                op=OP.subtract)
            fB = ep.tile([P, NBLK, 128], F32, tag="epB")
            nc.vector.tensor_tensor(out=fB[:], in0=fA[:], in1=fA[:],
                                    op=OP.mult)
            fss = wp.tile([P, NBLK], F32, tag="sc2")
            nc.vector.tensor_reduce(out=fss[:], in_=fB[:],
                                    axis=mybir.AxisListType.X, op=OP.add)
            fsd = wp.tile([P, NBLK], F32, tag="sc3")
            nc.scalar.activation(fsd[:], fss[:], AF.Sqrt, bias=eps_t[:],
                                 scale=1.0 / 128)
            nc.vector.reciprocal(fsd[:], fsd[:])
            nc.vector.tensor_tensor(
                out=fA[:], in0=fA[:], in1=_bap(fsd[:], [(1, NBLK), (0, 128)]),
                op=OP.mult)
            nc.vector.tensor_tensor(
                out=fA[:], in0=fA[:],
                in1=_bap(cc[:, CC_G2:CC_G2 + 128], [(0, NBLK), (1, 128)]),
                op=OP.mult)
            fG = ep.tile([P, NBLK, 128], BF, tag="epG")
            nc.vector.tensor_tensor(
                out=fG[:], in0=fA[:],
                in1=_bap(cc[:, CC_B2:CC_B2 + 128], [(0, NBLK), (1, 128)]),
                op=OP.add)
            nc.scalar.activation(fG[:], fG[:], AF.Gelu)

            # ---- output head + batched log_softmax ----
            for l in range(NBLK):
                hoT = wp.tile([P, P], BF, tag="xT")
                transpose_to(hoT[:], fG[:, l, :])
                pso_t = pss.tile([P, 130], F32, tag="mms")
                pso = pso_t[:, 0:D_OUT]
                nc.tensor.matmul(pso[:], lhsT=hoT[:], rhs=wos[:],
                                 start=True, stop=True)
                nc.vector.tensor_copy(out=z_all[:, l, :], in_=pso[:])

            nc.vector.tensor_tensor(
                out=z_all[:], in0=z_all[:],
                in1=_bap(cc[:, CC_BO:CC_BO + D_OUT], [(0, NBLK), (1, D_OUT)]),
                op=OP.add)
            zm = wp.tile([P, NBLK], F32, tag="sc0")
            nc.vector.tensor_reduce(out=zm[:], in_=z_all[:],
                                    axis=mybir.AxisListType.X, op=OP.max)
            nc.vector.tensor_tensor(
                out=z_all[:], in0=z_all[:],
                in1=_bap(zm[:], [(1, NBLK), (0, D_OUT)]), op=OP.subtract)
            ez = ep.tile([P, NBLK, D_OUT], F32, tag="epB")
            nc.scalar.activation(ez[:], z_all[:], AF.Exp)
            sden = wp.tile([P, NBLK], F32, tag="sc1")
            nc.vector.tensor_reduce(out=sden[:], in_=ez[:],
                                    axis=mybir.AxisListType.X, op=OP.add)
            lnd = wp.tile([P, NBLK], F32, tag="sc2")
            nc.scalar.activation(lnd[:], sden[:], AF.Ln)
            nc.vector.tensor_tensor(
                out=z_all[:], in0=z_all[:],
                in1=_bap(lnd[:], [(1, NBLK), (0, D_OUT)]), op=OP.subtract)
            nc.sync.dma_start(
                bass.AP(out.ap().tensor, 0,
                        [[D_OUT, P], [P * D_OUT, NBLK], [1, D_OUT]]),
                z_all[:])

    nc.compile()
    return nc


_CACHE = {}
_LAST_RUN = {}


def kernel(x, edge_index, g_in, b_in, W1, att1_s, att1_d, bias1, g1, b1,
           W2, att2_s, att2_d, bias2, g2, b2, Wo, bo):
    prep = prepare_inputs(x, edge_index)
    wts = prepare_weights(W1, att1_s, att1_d, bias1, g1, b1, g_in, b_in,
                          W2, att2_s, att2_d, bias2, g2, b2, Wo, bo)

    key = tuple(prep["ghat"])
    if key not in _CACHE:
        _CACHE[key] = build_program(prep["ghat"])
    nc = _CACHE[key]

    in_maps = []
    for c in range(NCORES):
        in_maps.append({
            "x_own": prep["x_own"][c],
            "idxw1": prep["idxw1"][c],
            "idxw2": prep["idxw2"][c],
            "alsfix": prep["alsfix"][c],
            "w1a8": wts["w1a8"],
            "w1f": wts["w1f"],
            "w2ext": wts["w2ext"].astype(BF16),
            "woext": wts["woext"],
            "colconst": wts["colconst"],
        })

    _LAST_RUN.update(nc=nc, in_maps=in_maps, prep=prep)
    res = bass_utils.run_bass_kernel_spmd(nc, in_maps,
                                          core_ids=list(range(NCORES)))
    outs = [res.results[c]["out"] for c in range(NCORES)]

    newid = prep["newid"]
    blk = newid // P
    core = blk % NCORES
    row = (blk // NCORES) * P + newid % P
    full = np.empty((N, D_OUT), dtype=np.float32)
    for c in range(NCORES):
        sel = core == c
        full[sel] = outs[c][row[sel]]
    return full
